# revision 1
# baseline (speedup 1.0000x reference)
"""Trainium2 Bass kernel for out = x * exclusive_cumsum(x, axis=time).

Input x: [B=8, T=4096, D=1024] f32. Pure data parallel: batch element b -> core b.

Per-core algorithm (x_c: [T, D], partition axis = time), group-pipelined:
  - T is split into 32 blocks of 128 rows, processed as 4 groups of 8 blocks.
  - Per block: one fp16 cast (ACT) feeds both passes below.
  - Totals: per block b = 8g+i, a colsum matmul with selector weights (ones in
    lhsT slice-column i) accumulates the block's column totals into row i of a
    group PSUM tile [8, 512] per 512-wide D chunk; one DVE copy per group drops
    them into rows [32g : 32g+8] of a shared fp16 totals tile [128, 512]
    (quadrant-aligned bases 0/32/64/96; gap rows stay zero via memset).
  - Per block: a strict-upper-triangular 128x128 matmul computes the
    within-block exclusive cumsum into PSUM (start=True); a second matmul with
    lhsT = wcar[0:32g+i, :] (wcar[k,m] = 1 iff k mod 32 < 8, so exactly the
    totals of blocks < b are summed; gap rows hit zero weights) adds the carry
    to every partition (start=False). DVE/ACT multiply f32 x by the f32 PSUM
    prefix; the result DMAs out.
  - Group g's compute starts as soon as its own totals copy lands, overlapping
    later groups' loads: the PE never waits on a global phase boundary.

All bulk DMA is linear 512KB blocks. PE matmuls run in fp16 (1 cycle/row);
all accumulation stays fp32 in PSUM.
"""

import sys

sys.path.insert(0, "/opt/trn_rl_repo")

import numpy as np

B, T, D = 8, 4096, 1024
BLK = 128
NBLK = T // BLK      # 32
GRP = 8              # blocks per group
NGRP = NBLK // GRP   # 4
NCH = 2
CH = D // NCH        # 512, exactly one PSUM bank in f32

_CACHE = {}


def _weights(np_dtype=np.float16):
    wtri = np.triu(np.ones((BLK, BLK), dtype=np_dtype), 1)  # [k,m]=1 iff k<m
    # Selector: ones in column 64 only; wsel[:, 64-i : 72-i] has ones exactly
    # in slice-column i.
    wsel = np.zeros((BLK, BLK), dtype=np_dtype)
    wsel[:, 64] = 1.0
    # Carry weights: row k is all-ones iff it is a real totals row (k mod 32
    # < GRP); sliced to [0:32g+i, :] it sums exactly the totals of blocks < b.
    k = np.arange(BLK)[:, None]
    wcar = ((k % 32) < GRP).astype(np_dtype) * np.ones((1, BLK), dtype=np_dtype)
    return wtri, wsel, wcar


def build_nc(t=T, d=D, nch=NCH, num_devices=B, early_copies=True):
    # early_copies: group-0 totals-prefix copies read finalized PSUM rows
    # while the accumulation group is still open. Verified correct on HW
    # (Tile orders copy_i between matmul_i and matmul_{i+1}; later matmuls
    # add exact zeros to rows <= i), but CoreSim forbids mid-group PSUM
    # reads, so the sim harness builds with early_copies=False.
    """Build the Bass module for one core's [t, d] shard."""
    import concourse.bass as bass
    import concourse.mybir as mybir
    import concourse.tile as tile
    from concourse import bacc

    f32 = mybir.dt.float32
    f16 = mybir.dt.float16
    ch = d // nch
    nblk = t // BLK
    ngrp = (nblk + GRP - 1) // GRP
    assert t % BLK == 0 and d % nch == 0 and ch <= 512 and nblk <= 32

    nc = bacc.Bacc("TRN2", target_bir_lowering=False, debug=False,
                   num_devices=num_devices)
    x = nc.dram_tensor("x", [t, d], f32, kind="ExternalInput").ap()
    wtri = nc.dram_tensor("wtri", [BLK, BLK], f16, kind="ExternalInput").ap()
    wsel = nc.dram_tensor("wsel", [BLK, BLK], f16, kind="ExternalInput").ap()
    wcar = nc.dram_tensor("wcar", [BLK, BLK], f16, kind="ExternalInput").ap()
    out = nc.dram_tensor("out", [t, d], f32, kind="ExternalOutput").ap()

    with tile.TileContext(nc) as tc:
        with (
            tc.tile_pool(name="wpool", bufs=1) as wpool,
            tc.tile_pool(name="xpool", bufs=16) as xpool,
            tc.tile_pool(name="hpool", bufs=12) as hpool,
            tc.tile_pool(name="spool", bufs=1) as spool,
            tc.tile_pool(name="opool", bufs=8) as opool,
            tc.tile_pool(name="ptot", bufs=1,
                         space=bass.MemorySpace.PSUM) as ptot,
            tc.tile_pool(name="pblk", bufs=3,
                         space=bass.MemorySpace.PSUM) as pblk,
        ):
            wt = wpool.tile([BLK, BLK], f16, tag="wt")
            nc.sync.dma_start(wt[:], wtri[:])
            ws = wpool.tile([BLK, BLK], f16, tag="ws")
            nc.sync.dma_start(ws[:], wsel[:])
            wc = wpool.tile([BLK, BLK], f16, tag="wc")
            nc.sync.dma_start(wc[:], wcar[:])

            totals = []
            for j in range(nch):
                tj = spool.tile([BLK, ch], f16, tag=f"tots{j}",
                                name=f"totals{j}")
                nc.vector.memset(tj[:], 0.0)
                totals.append(tj)

            for g in range(ngrp):
                blo = g * GRP
                bhi = min(blo + GRP, nblk)
                nb = bhi - blo

                xts, xas = [], []
                tot_psum = []
                for j in range(nch):
                    tp = ptot.tile([nb, ch], f32, tag=f"totg{j}",
                                   name=f"totg{g}_{j}")
                    tot_psum.append(tp)
                for i in range(nb):
                    b = blo + i
                    xt = xpool.tile([BLK, d], f32, tag="xt", name=f"xt{b}")
                    nc.sync.dma_start(xt[:], x[b * BLK:(b + 1) * BLK, :])
                    xts.append(xt)
                    xa = hpool.tile([BLK, d], f16, tag="xa", name=f"xa{b}")
                    nc.scalar.copy(xa[:], xt[:])
                    xas.append(xa)
                    for j in range(nch):
                        jc = slice(j * ch, (j + 1) * ch)
                        nc.tensor.matmul(
                            tot_psum[j][:],
                            ws[:, 64 - i:64 - i + nb],  # slice-col i only
                            xa[:, jc],
                            start=(i == 0), stop=(i == nb - 1),
                        )
                        if early_copies and g == 0 and i < nb - 1:
                            # Early prefix copy: rows 0..i are final (later
                            # selector matmuls add exact zeros there), so
                            # block i+1's carry unblocks without waiting for
                            # the whole group. Startup-critical group 0 only:
                            # extending this to all groups was measured SLOWER
                            # (DVE congestion + totals-tile WAR ping-pong).
                            nc.vector.tensor_copy(
                                totals[j][0:i + 1, :],
                                tot_psum[j][0:i + 1, :])
                for j in range(nch):
                    nc.vector.tensor_copy(
                        totals[j][32 * g:32 * g + nb, :], tot_psum[j][:])

                for i in range(nb):
                    b = blo + i
                    kb = 32 * g + i  # totals rows covering blocks < b
                    ot = opool.tile([BLK, d], f32, tag="out", name=f"ot{b}")
                    for j in range(nch):
                        jc = slice(j * ch, (j + 1) * ch)
                        ps = pblk.tile([BLK, ch], f32, tag=f"pb{j}",
                                       name=f"ps{b}_{j}")
                        nc.tensor.matmul(
                            ps[:], wt[:], xas[i][:, jc],
                            start=True, stop=(kb == 0),
                        )
                        if kb > 0:
                            nc.tensor.matmul(
                                ps[:],
                                wc[0:kb, :],         # rows k%32<8 are ones
                                totals[j][0:kb, :],
                                start=False, stop=True,
                            )
                        nc.any.tensor_mul(ot[:, jc], xts[i][:, jc],
                                          ps[:])
                        # Stores issue from the (otherwise idle) GpSimd
                        # sequencer so they never head-of-line-block later
                        # loads on sync; per-chunk so each starts as soon as
                        # its multiply lands.
                        nc.gpsimd.dma_start(
                            out[b * BLK:(b + 1) * BLK, jc], ot[:, jc])

    nc.compile()
    return nc


def kernel(x: np.ndarray) -> np.ndarray:
    from concourse.bass_utils import run_bass_kernel_spmd

    x = np.asarray(x, dtype=np.float32)
    assert x.shape == (B, T, D)
    key = "full"
    if key not in _CACHE:
        _CACHE[key] = build_nc()
    nc = _CACHE[key]

    wtri, wsel, wcar = _weights()
    in_maps = [
        {"x": np.ascontiguousarray(x[c]), "wtri": wtri, "wsel": wsel,
         "wcar": wcar}
        for c in range(B)
    ]
    res = run_bass_kernel_spmd(nc, in_maps, core_ids=list(range(B)))
    return np.stack([res.results[c]["out"] for c in range(B)], axis=0)

